# revision 4
# baseline (speedup 1.0000x reference)
"""Balanced BCE loss with per-sample dynamic top-k negative mining on 8 TRN2 cores.

Math: for each sample the reference computes
    pos_count = sum(gt*mask), neg_raw = sum((1-gt)*mask)
    neg_count = min(neg_raw, 3*pos_count), k = int(neg_count)
    loss = BCE(pred, gt);  pos_loss = sum(loss*positive)
    neg_topk = sum of k largest loss*negative values
    per_sample = (pos_loss + neg_topk) / (pos_count + neg_count + eps); mean over N.

Every negative position has loss > 0 (p is bounded away from {0,1}), so the
neg_loss vector has exactly neg_raw nonzero entries.  Whenever
neg_raw <= 3*pos_count, k == neg_raw and the top-k sum equals the FULL sum of
negative losses.  The device kernel therefore computes 4 streaming reductions
per sample:
    A = sum(gt*mask)            B = sum(mask - gt*mask)
    C = sum(gt*mask*ln(p))      D = sum((mask - gt*mask)*ln(1-p))
and the host combines 16x4 scalars.  If a sample ever violates
neg_raw <= 3*pos_count, the host recomputes that sample exactly (numpy).

Device mapping: data-parallel over N, 2 samples/core.  Each [640,640] sample
is viewed as [128, 3200]; per 1600-wide chunk: ScalarE computes ln(p) and
ln(1-p) (= Ln(-1*p + 1) via activation scale/bias) in bf16, VectorE casts
gt/mask to bf16 and forms the products (bf16 tensor_tensor runs in 2x mode),
TensorE reduces each product with a ones[128,1] stationary vector,
accumulating [1,400] column sums in PSUM across chunks.  Output [S,4,400]
partials are summed on the host in float64.  bf16 is exact for the 0/1
tensors and products with them, so the only bf16 rounding is on ln values
(~2^-9 relative per element, averaging out across ~100k elements).
"""

import sys

if "/opt/trn_rl_repo" not in sys.path:
    sys.path.insert(0, "/opt/trn_rl_repo")

import numpy as np

N, H, W = 16, 640, 640
NEG_RATIO = 3.0
EPS = 1e-8
N_CORES = 8
S = N // N_CORES          # samples per core
P = 128
FREE = H * W // P         # 3200
CHUNK = 1600              # DVE/ACT chunk (free dim)
NCHUNKS = FREE // CHUNK   # 2
MM = 400                  # matmul sub-chunk (PSUM bank: <=512 f32)
NMM = CHUNK // MM         # 4
NQ = 4                    # quantities A,B,C,D

_STATE = {}


def _build():
    import concourse.bass as bass
    import concourse.tile as tile
    from concourse import bacc, mybir

    f32 = mybir.dt.float32
    bf16 = mybir.dt.bfloat16
    Alu = mybir.AluOpType
    Act = mybir.ActivationFunctionType

    nc = bacc.Bacc("TRN2", target_bir_lowering=False, debug=False,
                   num_devices=N_CORES)
    pred_d = nc.dram_tensor("pred", [S, H, W], f32, kind="ExternalInput").ap()
    gt_d = nc.dram_tensor("gt", [S, H, W], f32, kind="ExternalInput").ap()
    mask_d = nc.dram_tensor("mask", [S, H, W], f32, kind="ExternalInput").ap()
    out_d = nc.dram_tensor("out", [S, NQ, MM], f32, kind="ExternalOutput").ap()

    with tile.TileContext(nc) as tc:
        with tc.tile_pool(name="cst", bufs=1) as cst, \
             tc.tile_pool(name="inp", bufs=3) as inp, \
             tc.tile_pool(name="mid", bufs=2) as mid, \
             tc.tile_pool(name="ps", bufs=2, space="PSUM") as psp:
            ones = cst.tile([P, 1], bf16)
            nc.gpsimd.memset(ones[:], 1.0)

            for s in range(S):
                pred_v = pred_d[s].rearrange("(p a) w -> p (a w)", p=P)
                gt_v = gt_d[s].rearrange("(p a) w -> p (a w)", p=P)
                mask_v = mask_d[s].rearrange("(p a) w -> p (a w)", p=P)
                accs = [psp.tile([1, MM], f32, tag=f"acc{q}",
                                 name=f"acc{q}_{s}")
                        for q in range(NQ)]
                for c in range(NCHUNKS):
                    sl = bass.ts(c, CHUNK)
                    tp = inp.tile([P, CHUNK], f32, tag="pred")
                    nc.sync.dma_start(tp[:], pred_v[:, sl])
                    tg = inp.tile([P, CHUNK], f32, tag="gt")
                    nc.sync.dma_start(tg[:], gt_v[:, sl])
                    tm = inp.tile([P, CHUNK], f32, tag="mask")
                    nc.sync.dma_start(tm[:], mask_v[:, sl])

                    lp = mid.tile([P, CHUNK], bf16, tag="lp")
                    nc.scalar.activation(lp[:], tp[:], Act.Ln)
                    l1p = mid.tile([P, CHUNK], bf16, tag="l1p")
                    nc.scalar.activation(l1p[:], tp[:], Act.Ln,
                                         bias=1.0, scale=-1.0)

                    g16 = mid.tile([P, CHUNK], bf16, tag="g16")
                    nc.vector.tensor_copy(g16[:], tg[:])
                    m16 = mid.tile([P, CHUNK], bf16, tag="m16")
                    nc.vector.tensor_copy(m16[:], tm[:])

                    gm = mid.tile([P, CHUNK], bf16, tag="gm")
                    nc.vector.tensor_tensor(gm[:], g16[:], m16[:], Alu.mult)
                    neg = mid.tile([P, CHUNK], bf16, tag="neg")
                    nc.vector.tensor_tensor(neg[:], m16[:], gm[:],
                                            Alu.subtract)
                    t1 = mid.tile([P, CHUNK], bf16, tag="t1")
                    nc.vector.tensor_tensor(t1[:], gm[:], lp[:], Alu.mult)
                    t2 = mid.tile([P, CHUNK], bf16, tag="t2")
                    nc.vector.tensor_tensor(t2[:], neg[:], l1p[:], Alu.mult)

                    for q, src in enumerate((gm, neg, t1, t2)):
                        for m in range(NMM):
                            step = c * NMM + m
                            nc.tensor.matmul(
                                accs[q][:], ones[:],
                                src[:, bass.ts(m, MM)],
                                start=(step == 0),
                                stop=(step == NCHUNKS * NMM - 1))

                for q in range(NQ):
                    ot = mid.tile([1, MM], f32, tag="ot",
                                  name=f"ot{q}_{s}")
                    nc.scalar.copy(ot[:], accs[q][:])
                    nc.sync.dma_start(out_d[s, q], ot[:])
    nc.compile()
    return nc


def _get_nc():
    if "nc" not in _STATE:
        _STATE["nc"] = _build()
    return _STATE["nc"]


def _host_topk_fallback(p, g, m):
    """Exact per-sample reference semantics in numpy (rare path)."""
    p = p.astype(np.float32)
    positive = g * m
    negative = (1.0 - g) * m
    pos_count = positive.sum(dtype=np.float64)
    neg_count = min(negative.sum(dtype=np.float64), pos_count * NEG_RATIO)
    log_p = np.maximum(np.log(p), -100.0)
    log_1mp = np.maximum(np.log1p(-p), -100.0)
    loss = -(g * log_p + (1.0 - g) * log_1mp)
    pos_loss_sum = (loss * positive).sum(dtype=np.float64)
    neg_loss = (loss * negative).ravel()
    k = int(neg_count)
    if k > 0:
        top = np.partition(neg_loss, len(neg_loss) - k)[len(neg_loss) - k:]
        neg_topk = top.sum(dtype=np.float64)
    else:
        neg_topk = 0.0
    return (pos_loss_sum + neg_topk) / (pos_count + neg_count + EPS)


def _combine(results, p, g, m):
    losses = []
    for c in range(N_CORES):
        o = results[c]["out"].astype(np.float64)  # [S, NQ, MM]
        sums = o.sum(axis=2)                      # [S, NQ]
        for s in range(S):
            A, B, C, D = sums[s]
            pos_count = A
            neg_raw = B
            neg_count = min(neg_raw, pos_count * NEG_RATIO)
            k = int(neg_count)
            if k >= int(round(neg_raw)):
                # top-k covers every (strictly positive) negative loss
                losses.append((-C - D) / (pos_count + neg_count + EPS))
            else:
                i = c * S + s
                losses.append(_host_topk_fallback(p[i], g[i], m[i]))
    return np.float32(np.mean(losses))


def kernel(pred, gt, mask):
    from concourse import bass_utils

    p = np.ascontiguousarray(pred[:, 0], dtype=np.float32)   # [N,H,W]
    g = np.ascontiguousarray(gt, dtype=np.float32)
    m = np.ascontiguousarray(mask, dtype=np.float32)

    nc = _get_nc()
    in_maps = [
        {"pred": p[c * S:(c + 1) * S],
         "gt": g[c * S:(c + 1) * S],
         "mask": m[c * S:(c + 1) * S]}
        for c in range(N_CORES)
    ]
    res = bass_utils.run_bass_kernel_spmd(nc, in_maps,
                                          core_ids=list(range(N_CORES)))
    return _combine(res.results, p, g, m)
